# revision 59
# baseline (speedup 1.0000x reference)
"""AtomPoolingLayer Trainium2 kernel (8 NeuronCores, data-parallel over molecules).

Reference computation (per molecule m of 512, atoms n=128, features f=512):
    w = sigmoid(relu(h @ W1 + b1) @ W2 + b2)        # gate, [M, N, 1]
    out[m, f] = sum_n w[m, n] * h[m, n, f]          # weighted pool, [M, F]

Sharding: h split on molecule dim across 8 cores (64 molecules/core); the tiny
MLP weights are replicated. No collectives needed.

Per-core pipeline (bf16 matmuls, DMA-bound target ~47us/core):
  DMA h (f32, natural [atom, mol, F]) -> GpSimd cast bf16 -> PE transpose
  (via identity) -> DVE copy PSUM->SBUF (hT) -> PE stage1 zT = W1.T @ hT ->
  ACT relu(+b1) -> PE stage2 w = zrelu.T @ W2 -> ACT sigmoid(+b2) ->
  PE stage3 out_m = w_m.T @ h_m -> ACT copy -> DMA out.

Engine assignment discipline: the Matmult ISA slot supports only ONE sync
wait, so every matmul may need at most one NEW cross-engine semaphore tick.
Hence: all hT copies on DVE, all activations/epilogue copies on ACT, cast +
identity setup on GpSimd, and every DMA-loaded constant is absorbed by a
copy on the engine that consumes it.
"""

import numpy as np

import concourse.bass as bass
import concourse.mybir as mybir
import concourse.tile as tile
from concourse.bass_utils import run_bass_kernel_spmd
from concourse.masks import make_identity

M, N, F = 512, 128, 512
HID = 128
N_CORES = 8
M_PER_CORE = M // N_CORES  # 64
G = 4  # molecules per pipeline group
N_GROUPS = M_PER_CORE // G
FP = mybir.dt.float32
FR = mybir.dt.float32r
BF = mybir.dt.bfloat16

_AF = mybir.ActivationFunctionType

_LAST_RESULTS = None


def build_bass():
    nc = bass.Bass()

    h_ext = nc.declare_dram_parameter("h", [M_PER_CORE, N, F], FP, isOutput=False)
    w1_ext = nc.declare_dram_parameter("W1", [F, HID], FP, isOutput=False)
    b1_ext = nc.declare_dram_parameter("b1", [HID], FP, isOutput=False)
    w2_ext = nc.declare_dram_parameter("W2", [HID, 1], FP, isOutput=False)
    b2_ext = nc.declare_dram_parameter("b2", [1], FP, isOutput=False)
    out_ext = nc.declare_dram_parameter("out", [M_PER_CORE, F], FP, isOutput=True)

    with tile.TileContext(nc) as tc:
        with (
            tc.tile_pool(name="singles", bufs=1) as singles,
            tc.tile_pool(name="hf32", bufs=4) as hf32p,
            tc.tile_pool(name="hbf", bufs=8) as hbfp,
            tc.tile_pool(name="ht", bufs=2) as htp,
            tc.tile_pool(name="zr", bufs=2) as zrp,
            tc.tile_pool(name="ps_t_v", bufs=2, space="PSUM") as pstp_v,
            tc.tile_pool(name="ps_t_s", bufs=2, space="PSUM") as pstp_s,
            tc.tile_pool(name="ps_z", bufs=2, space="PSUM") as pszp,
            tc.tile_pool(name="ps_w", bufs=1, space="PSUM") as pswp,
            tc.tile_pool(name="ps_o", bufs=1, space="PSUM") as psop,
        ):
            # ---------------- constants ----------------
            # identity (f32 gpsimd build, bf16 round on DVE for the transposes)
            ident_f32 = singles.tile([128, 128], FP)
            nc.gpsimd.memset(ident_f32, 0.0)
            ident_mk = nc.gpsimd.affine_select(
                out=ident_f32,
                in_=ident_f32,
                compare_op=mybir.AluOpType.not_equal,
                fill=1.0,
                base=0,
                pattern=[[-1, 128]],
                channel_multiplier=1,
            )
            ident = singles.tile([128, 128], BF)
            nc.vector.tensor_copy(ident, ident_f32)

            # W1 [F, HID] -> SBUF [k=128 (F within chunk), c=4 (F chunk), HID]
            w1f = singles.tile([128, 4, HID], FP)
            cdma1 = nc.gpsimd.dma_start(
                out=w1f, in_=w1_ext[:].rearrange("(c k) h -> k c h", k=128)
            )
            w1b = singles.tile([128, 4, HID], BF)
            nc.vector.tensor_copy(w1b, w1f)

            # b1 [HID] -> [128, 1] f32, absorbed through ACT (its consumer)
            b1raw = singles.tile([128, 1], FP)
            cdma2 = nc.gpsimd.dma_start(
                out=b1raw, in_=b1_ext[:].rearrange("(p o) -> p o", o=1)
            )
            b1s = singles.tile([128, 1], FP)
            nc.scalar.copy(b1s, b1raw)

            # W2 [HID, 1] -> bf16 [128, 1], cast on ACT (stage2 waits ACT)
            w2f = singles.tile([128, 1], FP)
            cdma3 = nc.gpsimd.dma_start(out=w2f, in_=w2_ext[:])
            w2b = singles.tile([128, 1], BF)
            nc.scalar.copy(w2b, w2f)

            # b2 [1] broadcast -> [128, 1] f32, absorbed through ACT
            b2raw = singles.tile([128, 1], FP)
            b2_bcast = bass.AP(tensor=b2_ext, offset=0, ap=[[0, 128], [1, 1]])
            cdma4 = nc.gpsimd.dma_start(out=b2raw, in_=b2_bcast)
            b2s = singles.tile([128, 1], FP)
            nc.scalar.copy(b2s, b2raw)

            # gate weights accumulate here: [atom, molecule] bf16
            w_sig = singles.tile([128, M_PER_CORE], BF)
            psum_w = pswp.tile([128, M_PER_CORE], FP)
            ps_o4 = psop.tile([128, F], FP)  # persistent stage-3 bank

            # output staging: molecule j of each group lands on partition 32j
            # (stage-3 col-tiling); one out-DMA per OB_BLOCK groups
            OB_BLOCK = 8
            ob4 = singles.tile([128, OB_BLOCK, F], FP)

            # probe scratch: disjoint columns, no probe-to-probe deps
            scr_dve = singles.tile([1, N_GROUPS], FP)
            scr_act = singles.tile([128, N_GROUPS + 3], FP)
            scr_act3 = singles.tile([1, N_GROUPS], FP)

            # one-time ACT probe past the constant copies
            nc.scalar.copy(scr_act[:, N_GROUPS + 2 : N_GROUPS + 3], b2s)

            h_view = h_ext[:]  # [M_PER_CORE, N, F]

            from concourse.bass import _add_dep_helper

            chains = {}

            def chained(key, inst):
                prev = chains.get(key)
                if prev is not None:
                    _add_dep_helper(
                        inst.ins, prev.ins, sync=False, reason=f"{key} order"
                    )
                chains[key] = inst
                return inst

            def pe(inst):
                return chained("pe", inst)

            def act(inst):
                return chained("act", inst)

            def dve(inst):
                return chained("dve", inst)

            HT_BUFS = 2
            hf_tiles = [None] * N_GROUPS
            hb_tiles = [None] * N_GROUPS
            ht_tiles = [None] * N_GROUPS
            s3_last = [None] * N_GROUPS
            htcopy_last = [None] * N_GROUPS
            obcopy_last = [None] * N_GROUPS
            outdma = [None] * N_GROUPS
            hfdma = [None] * N_GROUPS
            cast_inst = [None] * N_GROUPS

            def probe(chain_key, inst, dep):
                chained(chain_key, inst)
                _add_dep_helper(inst.ins, dep.ins, sync=True, reason="probe")
                return inst

            MOL_PER_LOAD = 16
            N_LOADS = M_PER_CORE // MOL_PER_LOAD  # 4
            GROUPS_PER_LOAD = MOL_PER_LOAD // G  # 4
            all_load_dmas = []

            def issue_hf(L, sizes=None, afters=None):
                # sizes: molecule counts per piece-DMA (same tile); pieces can
                # serialize behind prior DMAs (afters[k]) so early pieces get
                # full bandwidth during pipeline fill
                hf = hf32p.tile([128, MOL_PER_LOAD, F], FP, name=f"hf{L}", tag="hf")
                sizes = sizes or [MOL_PER_LOAD]
                pieces = []
                off = 0
                for k, sz in enumerate(sizes):
                    dma = nc.sync.dma_start(
                        out=hf[:, off : off + sz, :],
                        in_=h_view[
                            L * MOL_PER_LOAD + off : L * MOL_PER_LOAD + off + sz
                        ].rearrange("g n f -> n g f"),
                    )
                    if afters is not None and afters[k] is not None:
                        _add_dep_helper(
                            dma.ins, afters[k].ins, sync=True, reason="dma chain"
                        )
                    pieces.append(dma)
                    off += sz
                hfdma[L] = pieces[-1]
                hf_tiles[L] = hf
                all_load_dmas.extend(pieces)
                return pieces

            def issue_cast(gg):
                # cast h f32 -> bf16 on DVE
                hb = hbfp.tile([128, G, F], BF, name=f"hb{gg}", tag="hb")
                hb_tiles[gg] = hb
                gi = gg % GROUPS_PER_LOAD
                src = hf_tiles[gg // GROUPS_PER_LOAD][:, gi * G : (gi + 1) * G, :]
                cast_inst[gg] = dve(nc.vector.tensor_copy(hb, src))

            # pipeline fill: a tiny 4-molecule first piece gets the pipe
            # going ASAP; later pieces chain behind earlier ones
            p0 = issue_hf(0, sizes=[8, 8])
            p1 = issue_hf(1, sizes=[8, 8], afters=[p0[1], p0[1]])
            # prime casts two groups ahead
            issue_cast(0)
            issue_cast(1)

            zr_tiles = [None] * N_GROUPS
            act_iter_last = None  # last ACT inst of the previous iteration

            def front(g):
                # transposes + stage 1 + relu for group g
                hb = hb_tiles[g]
                if g >= HT_BUFS:
                    probe(
                        "dve",
                        nc.vector.memset(scr_dve[0:1, g : g + 1], 0.0),
                        htcopy_last[g - HT_BUFS],
                    )
                # copies split: molecules 0,1 -> DVE (ht_v), 2,3 -> ACT
                # (ht_s); separate transpose-psum pools per reader engine so
                # slot-reuse waits stay on one semaphore and get absorbed by
                # the matching stage-1 partial of the previous group
                ht_v = htp.tile([128, 4, 2, 128], BF, name=f"htv{g}", tag="htv")
                ht_s = htp.tile([128, 4, 2, 128], BF, name=f"hts{g}", tag="hts")
                ht_tiles[g] = ht_v
                ps_z = pszp.tile([128, G * 128], FP)

                def transpose_one(j):
                    pool_ = pstp_v if j < 2 else pstp_s
                    ps_t = pool_.tile(
                        [128, 4, 256], BF, name=f"pst{j}", tag=pool_.name
                    )
                    for c in range(4):
                        pe(
                            nc.tensor.transpose(
                                ps_t[:, c, :128],
                                hb[:, j, c * 128 : (c + 1) * 128],
                                ident,
                            )
                        )
                    if j < 2:
                        htcopy_last[g] = dve(
                            nc.vector.tensor_copy(
                                ht_v[:, :, j, :], ps_t[:, :, :128]
                            )
                        )
                    else:
                        act(
                            nc.scalar.copy(ht_s[:, :, j - 2, :], ps_t[:, :, :128])
                        )

                def stage1_half(h_idx):
                    src = ht_v if h_idx == 0 else ht_s
                    for c in range(4):
                        pe(
                            nc.tensor.matmul(
                                ps_z[:, h_idx * 256 : (h_idx + 1) * 256],
                                w1b[:, c, :],
                                src[:, c, :, :],
                                start=(c == 0),
                                stop=(c == 3),
                            )
                        )

                transpose_one(0)
                transpose_one(1)
                stage1_half(0)
                transpose_one(2)
                transpose_one(3)
                stage1_half(1)

                zr = zrp.tile([128, G * 128], BF, name=f"zr{g}", tag="zr")
                zr_tiles[g] = zr
                act(nc.scalar.activation(zr, ps_z, _AF.Relu, bias=b1s))

            def mid_stage(g):
                # stage 2 + sigmoid for group g
                zr = zr_tiles[g]
                for j in range(G):
                    mm = g * G + j
                    pe(
                        nc.tensor.matmul(
                            psum_w[:, mm : mm + 1],
                            zr[:, j * 128 : (j + 1) * 128],
                            w2b,
                            start=True,
                            stop=True,
                        )
                    )
                act(
                    nc.scalar.activation(
                        w_sig[:, g * G : (g + 1) * G],
                        psum_w[:, g * G : (g + 1) * G],
                        _AF.Sigmoid,
                        bias=b2s,
                    )
                )

            def back(g):
                # stage 3 + out staging + block DMA for group g
                hb = hb_tiles[g]
                if g % OB_BLOCK == 0 and g >= OB_BLOCK:
                    probe(
                        "act",
                        nc.scalar.mul(
                            scr_act3[0:1, g : g + 1], scr_act3[0:1, g : g + 1], 0.0
                        ),
                        outdma[g // OB_BLOCK - 1],
                    )
                for j in range(G):
                    mm = g * G + j
                    s3_last[g] = pe(
                        nc.tensor.matmul(
                            ps_o4[32 * j : 32 * j + 1, :],
                            w_sig[:, mm : mm + 1],
                            hb[:, j, :],
                            start=True,
                            stop=True,
                            tile_position=(0, 32 * j),
                        )
                    )
                obcopy_last[g] = act(
                    nc.scalar.copy(ob4[:, g % OB_BLOCK, :], ps_o4)
                )
                if g % OB_BLOCK == OB_BLOCK - 1:
                    blk = g // OB_BLOCK
                    outdma[blk] = nc.sync.dma_start(
                        out=out_ext[
                            blk * OB_BLOCK * G : (blk + 1) * OB_BLOCK * G
                        ].rearrange("(gi j) f -> j gi f", j=G),
                        in_=ob4[0:128:32, :, :],
                    )

            # depth-3 software pipeline: front(g) | mid(g-1) | back(g-2)
            for it in range(N_GROUPS + 2):
                g_f, g_m, g_b = it, it - 1, it - 2
                if g_f < N_GROUPS:
                    if (
                        g_f % GROUPS_PER_LOAD == GROUPS_PER_LOAD - 1
                        and g_f // GROUPS_PER_LOAD + 2 < N_LOADS
                    ):
                        Lx = g_f // GROUPS_PER_LOAD + 2
                        # L2 serialized behind L1's first piece; L3's slot
                        # release (cast of L0's last group) orders it anyway
                        issue_hf(Lx, afters=[p1[0]] if Lx == 2 else None)
                # ACT self-tick probe: past all of the previous iteration's ACT
                if act_iter_last is not None:
                    probe(
                        "act",
                        nc.scalar.mul(
                            scr_act[0:1, it : it + 1], scr_act[0:1, it : it + 1], 0.0
                        ),
                        act_iter_last,
                    )
                if g_f < N_GROUPS:
                    front(g_f)
                if 0 <= g_m < N_GROUPS:
                    mid_stage(g_m)
                if 0 <= g_b < N_GROUPS:
                    back(g_b)
                if g_f < N_GROUPS and g_f + 2 < N_GROUPS:
                    issue_cast(g_f + 2)
                act_iter_last = chains.get("act")

            # ---- tail: pre-advance SP's observed ticks so Tile's final drain
            # needs no waits of its own
            tail_deps = []
            tail_deps.extend(all_load_dmas)
            tail_deps.extend(outdma[: (N_GROUPS // OB_BLOCK)])
            tail_deps.extend([cdma1, cdma2, cdma3, cdma4])
            tail_deps.append(ident_mk)  # Pool
            tail_deps.append(chains["dve"])  # DVE
            tail_deps.append(obcopy_last[N_GROUPS - 1])  # ACT
            tail_deps.append(s3_last[N_GROUPS - 1])  # PE
            for dep in tail_deps:
                probe("sp", nc.sync.nop(nofuse=True, hint="tail_sink"), dep)

    return nc


_NC_CACHE = None


def kernel(h, W1, b1, W2, b2, _trace=False):
    global _NC_CACHE, _LAST_RESULTS
    h = np.ascontiguousarray(np.asarray(h, dtype=np.float32))
    W1 = np.ascontiguousarray(np.asarray(W1, dtype=np.float32))
    b1 = np.ascontiguousarray(np.asarray(b1, dtype=np.float32))
    W2 = np.ascontiguousarray(np.asarray(W2, dtype=np.float32))
    b2 = np.ascontiguousarray(np.asarray(b2, dtype=np.float32))

    if _NC_CACHE is None:
        _NC_CACHE = build_bass()
    nc = _NC_CACHE

    in_maps = []
    for i in range(N_CORES):
        in_maps.append(
            {
                "h": h[i * M_PER_CORE : (i + 1) * M_PER_CORE],
                "W1": W1,
                "b1": b1,
                "W2": W2,
                "b2": b2,
            }
        )

    res = run_bass_kernel_spmd(
        nc, in_maps, core_ids=list(range(N_CORES)), trace=_trace
    )
    _LAST_RESULTS = res
    out = np.concatenate([np.asarray(r["out"]) for r in res.results], axis=0)
    return out


# revision 61
# speedup vs baseline: 1.2418x; 1.2418x over previous
"""AtomPoolingLayer Trainium2 kernel (8 NeuronCores, data-parallel over molecules).

Reference computation (per molecule m of 512, atoms n=128, features f=512):
    w = sigmoid(relu(h @ W1 + b1) @ W2 + b2)        # gate, [M, N, 1]
    out[m, f] = sum_n w[m, n] * h[m, n, f]          # weighted pool, [M, F]

Sharding: h split on molecule dim across 8 cores (64 molecules/core); the tiny
MLP weights are replicated. No collectives needed.

Per-core pipeline (bf16 matmuls, DMA-bound target ~47us/core):
  DMA h (f32, natural [atom, mol, F]) -> GpSimd cast bf16 -> PE transpose
  (via identity) -> DVE copy PSUM->SBUF (hT) -> PE stage1 zT = W1.T @ hT ->
  ACT relu(+b1) -> PE stage2 w = zrelu.T @ W2 -> ACT sigmoid(+b2) ->
  PE stage3 out_m = w_m.T @ h_m -> ACT copy -> DMA out.

Engine assignment discipline: the Matmult ISA slot supports only ONE sync
wait, so every matmul may need at most one NEW cross-engine semaphore tick.
Hence: all hT copies on DVE, all activations/epilogue copies on ACT, cast +
identity setup on GpSimd, and every DMA-loaded constant is absorbed by a
copy on the engine that consumes it.
"""

import numpy as np

import concourse.bass as bass
import concourse.mybir as mybir
import concourse.tile as tile
from concourse.bass_utils import run_bass_kernel_spmd
from concourse.masks import make_identity

M, N, F = 512, 128, 512
HID = 128
N_CORES = 8
M_PER_CORE = M // N_CORES  # 64
G = 4  # molecules per pipeline group
N_GROUPS = M_PER_CORE // G
FP = mybir.dt.float32
FR = mybir.dt.float32r
BF = mybir.dt.bfloat16

_AF = mybir.ActivationFunctionType

_LAST_RESULTS = None


def build_bass():
    nc = bass.Bass()

    h_ext = nc.declare_dram_parameter("h", [M_PER_CORE, N, F], FP, isOutput=False)
    w1_ext = nc.declare_dram_parameter("W1", [F, HID], FP, isOutput=False)
    b1_ext = nc.declare_dram_parameter("b1", [HID], FP, isOutput=False)
    w2_ext = nc.declare_dram_parameter("W2", [HID, 1], FP, isOutput=False)
    b2_ext = nc.declare_dram_parameter("b2", [1], FP, isOutput=False)
    out_ext = nc.declare_dram_parameter("out", [M_PER_CORE, F], FP, isOutput=True)

    with tile.TileContext(nc) as tc:
        with (
            tc.tile_pool(name="singles", bufs=1) as singles,
            tc.tile_pool(name="hf32", bufs=4) as hf32p,
            tc.tile_pool(name="hbf", bufs=8) as hbfp,
            tc.tile_pool(name="ht", bufs=2) as htp,
            tc.tile_pool(name="zr", bufs=2) as zrp,
            tc.tile_pool(name="ps_t", bufs=3, space="PSUM") as pstp,
            tc.tile_pool(name="ps_z", bufs=2, space="PSUM") as pszp,
            tc.tile_pool(name="ps_w", bufs=1, space="PSUM") as pswp,
            tc.tile_pool(name="ps_o", bufs=2, space="PSUM") as psop,
        ):
            # ---------------- constants ----------------
            # identity (f32 gpsimd build, bf16 round on DVE for the transposes)
            ident_f32 = singles.tile([128, 128], FP)
            nc.gpsimd.memset(ident_f32, 0.0)
            ident_mk = nc.gpsimd.affine_select(
                out=ident_f32,
                in_=ident_f32,
                compare_op=mybir.AluOpType.not_equal,
                fill=1.0,
                base=0,
                pattern=[[-1, 128]],
                channel_multiplier=1,
            )
            ident = singles.tile([128, 128], BF)
            nc.vector.tensor_copy(ident, ident_f32)

            # W1 [F, HID] -> SBUF [k=128 (F within chunk), c=4 (F chunk), HID]
            w1f = singles.tile([128, 4, HID], FP)
            cdma1 = nc.gpsimd.dma_start(
                out=w1f, in_=w1_ext[:].rearrange("(c k) h -> k c h", k=128)
            )
            w1b = singles.tile([128, 4, HID], BF)
            nc.vector.tensor_copy(w1b, w1f)

            # b1 [HID] -> [128, 1] f32, absorbed through ACT (its consumer)
            b1raw = singles.tile([128, 1], FP)
            cdma2 = nc.gpsimd.dma_start(
                out=b1raw, in_=b1_ext[:].rearrange("(p o) -> p o", o=1)
            )
            b1s = singles.tile([128, 1], FP)
            nc.scalar.copy(b1s, b1raw)

            # W2 [HID, 1] -> bf16 [128, 1], cast on ACT (stage2 waits ACT)
            w2f = singles.tile([128, 1], FP)
            cdma3 = nc.gpsimd.dma_start(out=w2f, in_=w2_ext[:])
            w2b = singles.tile([128, 1], BF)
            nc.scalar.copy(w2b, w2f)

            # b2 [1] broadcast -> [128, 1] f32, absorbed through ACT
            b2raw = singles.tile([128, 1], FP)
            b2_bcast = bass.AP(tensor=b2_ext, offset=0, ap=[[0, 128], [1, 1]])
            cdma4 = nc.gpsimd.dma_start(out=b2raw, in_=b2_bcast)
            b2s = singles.tile([128, 1], FP)
            nc.scalar.copy(b2s, b2raw)

            # gate weights accumulate here: [atom, molecule] bf16
            w_sig = singles.tile([128, M_PER_CORE], BF)
            psum_w = pswp.tile([128, M_PER_CORE], FP)

            # output staging: molecule j of each group lands on partition 32j
            # (stage-3 col-tiling); one out-DMA per OB_BLOCK groups
            OB_BLOCK = 16
            ob4 = singles.tile([128, OB_BLOCK, F], FP)

            # probe scratch: disjoint columns, no probe-to-probe deps
            scr_dve = singles.tile([1, N_GROUPS], FP)
            scr_act = singles.tile([128, N_GROUPS + 3], FP)
            scr_act3 = singles.tile([1, N_GROUPS], FP)

            # one-time ACT probe past the constant copies
            nc.scalar.copy(scr_act[:, N_GROUPS + 2 : N_GROUPS + 3], b2s)

            h_view = h_ext[:]  # [M_PER_CORE, N, F]

            from concourse.bass import _add_dep_helper

            chains = {}

            def chained(key, inst):
                prev = chains.get(key)
                if prev is not None:
                    _add_dep_helper(
                        inst.ins, prev.ins, sync=False, reason=f"{key} order"
                    )
                chains[key] = inst
                return inst

            def pe(inst):
                return chained("pe", inst)

            def act(inst):
                return chained("act", inst)

            def dve(inst):
                return chained("dve", inst)

            HT_BUFS = 2
            hf_tiles = [None] * N_GROUPS
            hb_tiles = [None] * N_GROUPS
            ht_tiles = [None] * N_GROUPS
            s3_last = [None] * N_GROUPS
            htcopy_last = [None] * N_GROUPS
            obcopy_last = [None] * N_GROUPS
            outdma = [None] * N_GROUPS
            hfdma = [None] * N_GROUPS
            cast_inst = [None] * N_GROUPS

            def probe(chain_key, inst, dep):
                chained(chain_key, inst)
                _add_dep_helper(inst.ins, dep.ins, sync=True, reason="probe")
                return inst

            MOL_PER_LOAD = 16
            N_LOADS = M_PER_CORE // MOL_PER_LOAD  # 4
            GROUPS_PER_LOAD = MOL_PER_LOAD // G  # 4
            all_load_dmas = []

            def issue_hf(L, sizes=None, afters=None):
                # sizes: molecule counts per piece-DMA (same tile); pieces can
                # serialize behind prior DMAs (afters[k]) so early pieces get
                # full bandwidth during pipeline fill
                hf = hf32p.tile([128, MOL_PER_LOAD, F], FP, name=f"hf{L}", tag="hf")
                sizes = sizes or [MOL_PER_LOAD]
                pieces = []
                off = 0
                for k, sz in enumerate(sizes):
                    dma = nc.sync.dma_start(
                        out=hf[:, off : off + sz, :],
                        in_=h_view[
                            L * MOL_PER_LOAD + off : L * MOL_PER_LOAD + off + sz
                        ].rearrange("g n f -> n g f"),
                    )
                    if afters is not None and afters[k] is not None:
                        _add_dep_helper(
                            dma.ins, afters[k].ins, sync=True, reason="dma chain"
                        )
                    pieces.append(dma)
                    off += sz
                hfdma[L] = pieces[-1]
                hf_tiles[L] = hf
                all_load_dmas.extend(pieces)
                return pieces

            def issue_cast(gg):
                # cast h f32 -> bf16 on DVE
                hb = hbfp.tile([128, G, F], BF, name=f"hb{gg}", tag="hb")
                hb_tiles[gg] = hb
                gi = gg % GROUPS_PER_LOAD
                src = hf_tiles[gg // GROUPS_PER_LOAD][:, gi * G : (gi + 1) * G, :]
                cast_inst[gg] = dve(nc.vector.tensor_copy(hb, src))

            # pipeline fill: a tiny 4-molecule first piece gets the pipe
            # going ASAP; later pieces chain behind earlier ones
            p0 = issue_hf(0, sizes=[8, 8])
            p1 = issue_hf(1, sizes=[8, 8], afters=[p0[1], p0[1]])
            # prime casts two groups ahead
            issue_cast(0)
            issue_cast(1)

            zr_tiles = [None] * N_GROUPS
            act_iter_last = None  # last ACT inst of the previous iteration

            def front(g):
                # transposes + stage 1 + relu for group g
                hb = hb_tiles[g]
                if g >= HT_BUFS:
                    probe(
                        "dve",
                        nc.vector.memset(scr_dve[0:1, g : g + 1], 0.0),
                        htcopy_last[g - HT_BUFS],
                    )
                ht = htp.tile([128, 4, G, 128], BF, name=f"ht{g}", tag="ht")
                ht_tiles[g] = ht
                ps_z = pszp.tile([128, G * 128], FP)

                def transpose_one(j):
                    ps_t = pstp.tile([128, 4, 256], BF)
                    for c in range(4):
                        pe(
                            nc.tensor.transpose(
                                ps_t[:, c, :128],
                                hb[:, j, c * 128 : (c + 1) * 128],
                                ident,
                            )
                        )
                    htcopy_last[g] = dve(
                        nc.vector.tensor_copy(ht[:, :, j, :], ps_t[:, :, :128])
                    )

                def stage1_half(h_idx):
                    for c in range(4):
                        pe(
                            nc.tensor.matmul(
                                ps_z[:, h_idx * 256 : (h_idx + 1) * 256],
                                w1b[:, c, :],
                                ht[:, c, 2 * h_idx : 2 * h_idx + 2, :],
                                start=(c == 0),
                                stop=(c == 3),
                            )
                        )

                transpose_one(0)
                transpose_one(1)
                stage1_half(0)
                transpose_one(2)
                transpose_one(3)
                stage1_half(1)

                zr = zrp.tile([128, G * 128], BF, name=f"zr{g}", tag="zr")
                zr_tiles[g] = zr
                act(nc.scalar.activation(zr, ps_z, _AF.Relu, bias=b1s))

            def mid_stage(g):
                # stage 2 + sigmoid for group g
                zr = zr_tiles[g]
                for j in range(G):
                    mm = g * G + j
                    pe(
                        nc.tensor.matmul(
                            psum_w[:, mm : mm + 1],
                            zr[:, j * 128 : (j + 1) * 128],
                            w2b,
                            start=True,
                            stop=True,
                        )
                    )
                act(
                    nc.scalar.activation(
                        w_sig[:, g * G : (g + 1) * G],
                        psum_w[:, g * G : (g + 1) * G],
                        _AF.Sigmoid,
                        bias=b2s,
                    )
                )

            def back(g):
                # stage 3 + out staging + block DMA for group g
                hb = hb_tiles[g]
                if g % OB_BLOCK == 0 and g >= OB_BLOCK:
                    probe(
                        "act",
                        nc.scalar.mul(
                            scr_act3[0:1, g : g + 1], scr_act3[0:1, g : g + 1], 0.0
                        ),
                        outdma[g // OB_BLOCK - 1],
                    )
                ps_o4 = psop.tile([128, F], FP)
                for j in range(G):
                    mm = g * G + j
                    s3_last[g] = pe(
                        nc.tensor.matmul(
                            ps_o4[32 * j : 32 * j + 1, :],
                            w_sig[:, mm : mm + 1],
                            hb[:, j, :],
                            start=True,
                            stop=True,
                            tile_position=(0, 32 * j),
                        )
                    )
                obcopy_last[g] = act(
                    nc.scalar.copy(ob4[:, g % OB_BLOCK, :], ps_o4)
                )
                if g % OB_BLOCK == OB_BLOCK - 1:
                    blk = g // OB_BLOCK
                    outdma[blk] = nc.sync.dma_start(
                        out=out_ext[
                            blk * OB_BLOCK * G : (blk + 1) * OB_BLOCK * G
                        ].rearrange("(gi j) f -> j gi f", j=G),
                        in_=ob4[0:128:32, :, :],
                    )

            # depth-3 software pipeline: front(g) | mid(g-1) | back(g-2)
            for it in range(N_GROUPS + 2):
                g_f, g_m, g_b = it, it - 1, it - 2
                if g_f < N_GROUPS:
                    if (
                        g_f % GROUPS_PER_LOAD == GROUPS_PER_LOAD - 1
                        and g_f // GROUPS_PER_LOAD + 2 < N_LOADS
                    ):
                        Lx = g_f // GROUPS_PER_LOAD + 2
                        # L2 split across two queues (a single-piece 4MB DMA
                        # runs ~18us at per-queue rate and stalls the g=8
                        # block boundary); L3 has 30us of slack, keep single
                        if Lx == 2:
                            issue_hf(Lx, sizes=[8, 8], afters=[p1[0], p1[1]])
                        else:
                            issue_hf(Lx)
                # ACT self-tick probe: past all of the previous iteration's ACT
                if act_iter_last is not None:
                    probe(
                        "act",
                        nc.scalar.mul(
                            scr_act[0:1, it : it + 1], scr_act[0:1, it : it + 1], 0.0
                        ),
                        act_iter_last,
                    )
                if g_f < N_GROUPS:
                    front(g_f)
                if 0 <= g_m < N_GROUPS:
                    mid_stage(g_m)
                if 0 <= g_b < N_GROUPS:
                    back(g_b)
                if g_f < N_GROUPS and g_f + 2 < N_GROUPS:
                    issue_cast(g_f + 2)
                act_iter_last = chains.get("act")

            # ---- tail: pre-advance SP's observed ticks so Tile's final drain
            # needs no waits of its own
            tail_deps = []
            tail_deps.extend(all_load_dmas)
            tail_deps.extend(outdma[: (N_GROUPS // OB_BLOCK)])
            tail_deps.extend([cdma1, cdma2, cdma3, cdma4])
            tail_deps.append(ident_mk)  # Pool
            tail_deps.append(chains["dve"])  # DVE
            tail_deps.append(obcopy_last[N_GROUPS - 1])  # ACT
            tail_deps.append(s3_last[N_GROUPS - 1])  # PE
            for dep in tail_deps:
                probe("sp", nc.sync.nop(nofuse=True, hint="tail_sink"), dep)

    return nc


_NC_CACHE = None


def kernel(h, W1, b1, W2, b2, _trace=False):
    global _NC_CACHE, _LAST_RESULTS
    h = np.ascontiguousarray(np.asarray(h, dtype=np.float32))
    W1 = np.ascontiguousarray(np.asarray(W1, dtype=np.float32))
    b1 = np.ascontiguousarray(np.asarray(b1, dtype=np.float32))
    W2 = np.ascontiguousarray(np.asarray(W2, dtype=np.float32))
    b2 = np.ascontiguousarray(np.asarray(b2, dtype=np.float32))

    if _NC_CACHE is None:
        _NC_CACHE = build_bass()
    nc = _NC_CACHE

    in_maps = []
    for i in range(N_CORES):
        in_maps.append(
            {
                "h": h[i * M_PER_CORE : (i + 1) * M_PER_CORE],
                "W1": W1,
                "b1": b1,
                "W2": W2,
                "b2": b2,
            }
        )

    res = run_bass_kernel_spmd(
        nc, in_maps, core_ids=list(range(N_CORES)), trace=_trace
    )
    _LAST_RESULTS = res
    out = np.concatenate([np.asarray(r["out"]) for r in res.results], axis=0)
    return out
